# revision 17
# baseline (speedup 1.0000x reference)
"""Trainium2 Bass kernel for EnhancedLocalAttentionWithGQA (differential
windowed attention, B=2 L=4096 E=1024 H=16 G=2 W=256 D=64).

Key structural facts exploited:
  - The reference concatenates nw=31 overlapping windows along the sequence
    and trims to L=4096 = 16*W, so ONLY windows 0..15 contribute, and only
    input positions 0..2175 are used as queries/keys/values.
  - Output row p comes from window n = p//256, in-window query j = p%256,
    i.e. input position n*128 + j.

Sharding: 8 cores, core c owns windows (2c, 2c+1) -> output rows
[512c, 512c+512) for both batches. Each core needs x rows [256c, 256c+384).

v2: whole matmul pipeline in bf16 (error budget 2e-2, measured ~6.5e-3);
fp32r small-N matmuls ran 4 cycles/row + slow LDWEIGHTS, bf16 is 1
cycle/row.  All weights HBM-resident in bf16 in the exact SBUF strip
layout (fully contiguous per-partition DMA).        [436852 -> 236862 ns]

v3: keep the PE p-state at full clock by never letting it idle: batch-1
projections are emitted as filler between batch-0 attention heads, score
matmuls are software-pipelined one head ahead of PV, rope's second
multiply and the t2 scale move to the idle GpSimd engine, and the first
xt/wq DMAs are split into chunks so the first projection starts ~8us
earlier.

On-device dataflow (per core, SPMD — all core differences come via inputs):
  - q^T / k^T computed directly in [head-dim, seq] layout (lhsT = weight
    tile, rhs = x^T tile). Host pre-permutes W columns so each head block
    is [evens | odds], making RoPE 2 full-tile muls + 4 partition-offset
    add/subs. Branch-2 weights are block-swapped so the two differential
    branches occupy complementary 64-partition halves (concurrent K=64
    score matmuls via PE row groups).
  - Scores computed transposed: S^T[k, q] (lhsT = k^T, rhs = q^T), exp on
    ACT without max-subtraction (scores are small), then PV as
    out[q, 65] = E_tile.T @ [v | ones-ish] with an extra column giving the
    softmax denominator (branch 2 uses 1/lambda so its reciprocal is
    lambda/den2).
  - Normalize+combine: one GpSimd scale (t2 = num2*r2) + one DVE
    scalar_tensor_tensor -> a = num1*r1 - t2.
  - a transposed back via PE transpose (head pair packed into one PSUM
    tile), out-projection accumulated over 8 K-tiles + bias via K=1 matmul.
"""

import os
import sys

sys.path.insert(0, "/opt/trn_rl_repo")
os.environ.setdefault("MYCRO_LOCAL_CACHE", "1")

import numpy as np
import ml_dtypes

BF16 = ml_dtypes.bfloat16

B, L, E, H, G, W, D = 2, 4096, 1024, 16, 2, 256, 64
NCORES = 8
SEQ = 384          # x rows per core
NW = 2             # windows per core
QROWS = 512        # output rows per core per batch
KV = E // (H // G)  # 128
LAMBDA_INIT = 0.8


# ----------------------------------------------------------------- host prep

def _head_perm():
    """Column permutation applied to Wq1/Wk1: per 64-block -> [evens|odds]."""
    p = []
    for blk in range(0, E, D):
        p += [blk + 2 * j for j in range(D // 2)]
        p += [blk + 2 * j + 1 for j in range(D // 2)]
    return np.array(p, dtype=np.int64)


def _q2_perm():
    """q2: like _head_perm but heads swapped within each 128-col M-tile."""
    base = _head_perm()
    p = np.empty_like(base)
    for m in range(E // 128):
        p[m * 128: m * 128 + 64] = base[m * 128 + 64: m * 128 + 128]
        p[m * 128 + 64: m * 128 + 128] = base[m * 128: m * 128 + 64]
    return p


def _k_perm(swap):
    """kv columns (128 = 2 groups x 64): per group block [evens|odds];
    swap=True puts group1 first (branch-2 layout)."""
    p = []
    groups = (1, 0) if swap else (0, 1)
    for g in groups:
        blk = g * D
        p += [blk + 2 * j for j in range(D // 2)]
        p += [blk + 2 * j + 1 for j in range(D // 2)]
    return np.array(p, dtype=np.int64)


def _strip_w(w):
    """(1024, M) -> (M//128 mt, 128 p, 8k*128) strips: strip[mt][p][k*128+s]
    = w[k*128+p, mt*128+s]; per-partition rows are fully contiguous."""
    kdim = w.shape[0] // 128
    mdim = w.shape[1] // 128
    t = w.reshape(kdim, 128, mdim, 128).transpose(2, 1, 0, 3)
    return np.ascontiguousarray(t.reshape(mdim, 128, kdim * 128))


def _trig_tables(core):
    pos = (256 * core + np.arange(SEQ, dtype=np.float64))  # global positions
    div = np.exp(np.arange(0, D, 2, dtype=np.float64) * (-np.log(10000.0) / D))
    ang = pos[None, :] * div[:, None]          # (32, SEQ)
    c32 = np.cos(ang).astype(np.float32)
    s32 = np.sin(ang).astype(np.float32)
    tc = np.tile(c32, (4, 1))                   # (128, SEQ)
    # sign-folded sin: rows [0:32]=+sin (qe*sin for the odd half),
    # [32:64]=-sin (-qo*sin for the even half), repeating per 64-block.
    tsn = np.tile(np.concatenate([s32, -s32], axis=0), (2, 1))
    return np.ascontiguousarray(tc), np.ascontiguousarray(tsn)


def _p32():
    """[128,128] permutation: swaps 32-halves within each 64-block.
    Used as matmul lhsT: out = P.T @ u with P[k, m] = 1 iff k = swap(m)."""
    p = np.zeros((128, 128), np.float32)
    for m in range(128):
        k = m + 32 if (m % 64) < 32 else m - 32
        p[k, m] = 1.0
    return p


# ------------------------------------------------------------ device program

_PROGRAM_CACHE = {}


def _build_program():
    import concourse.bass as bass
    import concourse.mybir as mybir
    import concourse.tile as tile
    from concourse.masks import make_identity
    from concourse.tile_rust import add_dep_helper

    def order_group(insts):
        """PE-order a bank-packed accumulation group: first (start=True)
        before everything, last (stop=True) after everything. sync=False —
        same-engine ordering only."""
        first, last = insts[0], insts[-1]
        for i in insts[1:]:
            add_dep_helper(i.ins, first.ins, sync=False,
                           reason="psum group start first")
        for i in insts[:-1]:
            add_dep_helper(last.ins, i.ins, sync=False,
                           reason="psum group stop last")

    f32 = mybir.dt.float32
    f32r = mybir.dt.float32r
    bf16 = mybir.dt.bfloat16
    ALU = mybir.AluOpType
    ACTF = mybir.ActivationFunctionType

    nc = bass.Bass()

    xt_d = nc.dram_tensor("xt", [B, 128, 8 * SEQ], bf16, kind="ExternalInput")
    wq_d = nc.dram_tensor("wq", [2, 8, 128, 1024], bf16, kind="ExternalInput")
    wk_d = nc.dram_tensor("wk", [2, 128, 1024], bf16, kind="ExternalInput")
    wv_d = nc.dram_tensor("wv", [128, 1024], bf16, kind="ExternalInput")
    wo_d = nc.dram_tensor("wo", [8, 128, 1024], bf16, kind="ExternalInput")
    tc_d = nc.dram_tensor("tct", [128, SEQ], f32, kind="ExternalInput")
    ts_d = nc.dram_tensor("tst", [128, SEQ], f32, kind="ExternalInput")
    lam_d = nc.dram_tensor("lamv", [128, 2], f32, kind="ExternalInput")
    bout_d = nc.dram_tensor("boutv", [1, E], f32r, kind="ExternalInput")
    p32_d = nc.dram_tensor("p32", [128, 128], bf16, kind="ExternalInput")
    ones_d = nc.dram_tensor("onesv", [1, 128], f32r, kind="ExternalInput")
    y_d = nc.dram_tensor("y", [B, QROWS, E], f32, kind="ExternalOutput")

    def split_matmul_waits():
        """This walrus build allows only ONE sync-wait per engine
        instruction; peel extra waits onto engine-matched no-ops placed
        just before the instruction."""
        for bb in nc.m.functions[0].blocks:
            il = bb.instructions
            new_list = []
            changed = False
            for i in il:
                si = getattr(i, "sync_info", None)
                if si is not None and len(si.on_wait) > 1:
                    waits = list(si.on_wait)
                    for j, w in enumerate(waits[1:]):
                        nop = mybir.InstNoOp(
                            name=f"{i.name}-wnop{j}", engine=i.engine, ins=[],
                            outs=[],
                            sync_info=mybir.SyncInfo(on_wait=[w],
                                                     on_update=[]))
                        nc.inst_map[nop.name] = nop
                        new_list.append(nop)
                    i.sync_info = mybir.SyncInfo(
                        on_wait=[waits[0]], on_update=list(si.on_update))
                    changed = True
                new_list.append(i)
            if changed:
                il[:] = new_list
    with tile.TileContext(nc) as tc:
        with tc.tile_pool(name="const", bufs=1) as constp, \
             tc.tile_pool(name="xt", bufs=1) as xtp, \
             tc.tile_pool(name="rot", bufs=1) as rotp, \
             tc.tile_pool(name="wres", bufs=1) as wresp, \
             tc.tile_pool(name="ru", bufs=3) as rup, \
             tc.tile_pool(name="vext", bufs=1) as vxp, \
             tc.tile_pool(name="att", bufs=4) as attp, \
             tc.tile_pool(name="small", bufs=3) as smp, \
             tc.tile_pool(name="pairs", bufs=6) as pairp, \
             tc.tile_pool(name="atile", bufs=2) as atp, \
             tc.tile_pool(name="psA", bufs=2, space="PSUM") as psA, \
             tc.tile_pool(name="psSC", bufs=4, space="PSUM") as psSC, \
             tc.tile_pool(name="psPV", bufs=2, space="PSUM") as psPV:

            # constants
            ones1 = constp.tile([1, 128], f32r, tag="ones1")
            nc.sync.dma_start(out=ones1, in_=ones_d[:, :])
            p32_sb = constp.tile([128, 128], bf16, tag="p32s")
            nc.sync.dma_start(out=p32_sb, in_=p32_d[:, :])
            tc_sb = constp.tile([128, SEQ], f32, tag="tcs")
            ts_sb = constp.tile([128, SEQ], f32, tag="tss")
            lam_sb = constp.tile([128, 2], f32, tag="lams")
            bout_sb = constp.tile([1, E], f32r, tag="bouts")
            nc.sync.dma_start(out=tc_sb, in_=tc_d[:, :])
            nc.sync.dma_start(out=ts_sb, in_=ts_d[:, :])
            nc.sync.dma_start(out=lam_sb, in_=lam_d[:, :])
            nc.sync.dma_start(out=bout_sb, in_=bout_d[:, :])

            # x^T strips.  b0's strip and the first q weight strip are
            # split into chunks and interleaved so the very first
            # projection group can start after ~1/4 of the data landed.
            xts = {}
            strips = {}
            for b in range(B):
                strips[b] = xtp.tile([128, 8 * SEQ], bf16, tag=f"xt{b}",
                                     name=f"xt{b}")
                for kt in range(8):
                    xts[b, kt] = strips[b][:, kt * SEQ:(kt + 1) * SEQ]
            wq_sb = {}
            wq_sb[0, 0] = wresp.tile([128, 1024], bf16, tag="wq_0_0",
                                     name="wq_0_0")
            for j in range(4):
                nc.sync.dma_start(out=strips[0][:, j * 768:(j + 1) * 768],
                                  in_=xt_d[0, :, j * 768:(j + 1) * 768])
                nc.sync.dma_start(
                    out=wq_sb[0, 0][:, j * 256:(j + 1) * 256],
                    in_=wq_d[0, 0, :, j * 256:(j + 1) * 256])
            # resident weights, bf16, contiguous strips.  The next few q
            # strips go before xt batch-1 (they gate the projection
            # pipeline; batch-1 data is not needed until the attention
            # phase).
            for mat in range(2):
                for mt in range(8):
                    if (mat, mt) == (0, 0):
                        continue
                    t = wresp.tile([128, 1024], bf16, tag=f"wq_{mat}_{mt}")
                    nc.sync.dma_start(out=t, in_=wq_d[mat, mt, :, :])
                    wq_sb[mat, mt] = t
                    if (mat, mt) == (0, 4):
                        nc.sync.dma_start(out=strips[1], in_=xt_d[1, :, :])
            wk_sb = {}
            for mat in range(2):
                wk_sb[mat] = wresp.tile([128, 1024], bf16, tag=f"wk_{mat}",
                                        name=f"wk_{mat}")
                nc.sync.dma_start(out=wk_sb[mat], in_=wk_d[mat, :, :])
            wv_sb = wresp.tile([128, 1024], bf16, tag="wv")
            nc.sync.dma_start(out=wv_sb, in_=wv_d[:, :])
            wo_sb = {}
            for kt in range(8):
                t = wresp.tile([128, 1024], bf16, tag=f"wo{kt}")
                nc.sync.dma_start(out=t, in_=wo_d[kt, :, :])
                for nh in range(2):
                    wo_sb[kt, nh] = t[:, nh * 512:(nh + 1) * 512]

            qrot = {}
            krot = {}
            vext = {}

            def rope(psum_in, rot_out):
                # rot = psum*TC + P32 @ (psum*TS_signed)
                # (the PE matmul does the cross-partition 32-half swap that
                # DVE cannot: walrus requires same start partition on all
                # InstTensorTensor operands; GpSimd cannot access PSUM)
                t = rup.tile([128, SEQ], bf16, tag="ropet")
                u = rup.tile([128, SEQ], bf16, tag="ropeu")
                nc.vector.tensor_mul(t, psum_in, tc_sb)
                nc.vector.tensor_mul(u, psum_in, ts_sb)
                usw = psSC.tile([128, 512], f32, tag="sc")
                nc.tensor.matmul(usw[:, 0:SEQ], p32_sb,
                                 u, start=True, stop=True)
                nc.vector.tensor_add(rot_out, t, usw[:, 0:SEQ])

            def emit_qproj(mat, mt, b):
                ps = psA.tile([128, 512], f32, tag="proj")
                qp = ps[:, 0:SEQ]
                for kt in range(8):
                    nc.tensor.matmul(
                        qp,
                        wq_sb[mat, mt][:, kt * 128:(kt + 1) * 128],
                        xts[b, kt],
                        start=(kt == 0), stop=(kt == 7))
                rot = rotp.tile([128, SEQ], bf16, tag=f"q{mat}_{b}_{mt}",
                                name=f"q{mat}_{b}_{mt}")
                rope(qp, rot)
                qrot[mat, b, mt] = rot

            def emit_kproj(mat, b):
                ps = psA.tile([128, 512], f32, tag="proj")
                kp = ps[:, 0:SEQ]
                for kt in range(8):
                    nc.tensor.matmul(
                        kp, wk_sb[mat][:, kt * 128:(kt + 1) * 128],
                        xts[b, kt],
                        start=(kt == 0), stop=(kt == 7))
                rot = rotp.tile([128, SEQ], bf16, tag=f"k{mat}_{b}",
                                name=f"k{mat}_{b}")
                rope(kp, rot)
                krot[mat, b] = rot

            def emit_vproj(b, st):
                ps = psA.tile([128, 512], f32, tag="proj")
                vp = ps[:, 0:128]
                for kt in range(8):
                    nc.tensor.matmul(
                        vp,
                        xts[b, kt][:, st * 128:(st + 1) * 128],
                        wv_sb[:, kt * 128:(kt + 1) * 128],
                        start=(kt == 0), stop=(kt == 7))
                for g in range(2):
                    for ver in range(2):
                        ve = vxp.tile([128, 65], bf16,
                                      tag=f"ve{ver}_{b}_{st}_{g}",
                                      name=f"ve{ver}_{b}_{st}_{g}")
                        nc.vector.tensor_copy(ve[:, 0:64],
                                              vp[:, g * 64:(g + 1) * 64])
                        nc.gpsimd.tensor_copy(ve[:, 64:65],
                                              lam_sb[:, ver:ver + 1])
                        vext[ver, b, st, g] = ve

            # ---- batch-0 projections ----
            for mat in range(2):
                for mt in range(8):
                    emit_qproj(mat, mt, 0)
            for mat in range(2):
                emit_kproj(mat, 0)
            for st in range(3):
                emit_vproj(0, st)

            # batch-1 projection thunks: emitted as PE filler between
            # batch-0 attention heads (keeps the PE p-state at full clock
            # and moves their DVE rope work into the ACT-heavy phase).
            fillers = [(lambda mat=mat, mt=mt: emit_qproj(mat, mt, 1))
                       for mat in range(2) for mt in range(8)]
            fillers += [(lambda mat=mat: emit_kproj(mat, 1))
                        for mat in range(2)]
            fillers += [(lambda st=st: emit_vproj(1, st)) for st in range(3)]
            fillers.reverse()  # pop() order = original order

            # ---- attention + output projection ----
            def emit_scores(b, w, h):
                g = h & 1
                mt = h >> 1
                base1 = 64 * g          # branch-1 partition base
                base2 = 64 - base1      # branch-2 partition base
                e_sb = []
                for br, qb in ((0, base1), (1, base2)):
                    st_ps = psSC.tile([128, 512], f32, tag="sc")
                    sc_mms = []
                    for kts in range(2):
                        sc_mms.append(nc.tensor.matmul(
                            st_ps[:, kts * 256:(kts + 1) * 256],
                            krot[br, b][qb:qb + 64,
                                        w * 128 + kts * 128:
                                        w * 128 + kts * 128 + 128]
                            ,
                            qrot[br, b, mt][qb:qb + 64,
                                            w * 128:w * 128 + 256]
                            ,
                            start=(kts == 0), stop=(kts == 1)))
                    order_group(sc_mms)
                    e = attp.tile([128, 512], bf16, tag=f"e{br}")
                    nc.scalar.activation(e, st_ps, ACTF.Exp,
                                         scale=0.125)
                    e_sb.append(e)
                return e_sb

            def make_y_thunk(b, w, qt, nh, at_map):
                def thunk():
                    y_ps = psA.tile([128, 512], f32, tag="proj")
                    y_mms = []
                    for kt in range(8):
                        y_mms.append(nc.tensor.matmul(
                            y_ps, at_map[kt, qt],
                            wo_sb[kt, nh],
                            start=(kt == 0), stop=False))
                    y_mms.append(nc.tensor.matmul(
                        y_ps, ones1,
                        bout_sb[:, nh * 512:(nh + 1) * 512],
                        start=False, stop=True))
                    order_group(y_mms)
                    y_sb = smp.tile([128, 512], f32, tag="ysb")
                    if nh == 0:
                        nc.scalar.activation(y_sb, y_ps, ACTF.Copy)
                    else:
                        nc.vector.tensor_copy(y_sb, y_ps)
                    nc.sync.dma_start(
                        out=y_d[b, (w * 2 + qt) * 128:
                                (w * 2 + qt) * 128 + 128,
                                nh * 512:(nh + 1) * 512],
                        in_=y_sb)
                return thunk

            # y-projection groups for block (b,w) are emitted as PE filler
            # between the NEXT block's attention heads: they are pure
            # tensor-engine work that plugs the per-head pipeline gaps and
            # keeps the PE p-state at full clock.
            prev_y = []
            for b in range(B):
                for w in range(NW):
                    at_sb = {}
                    pair_sb = None
                    e_pipe = emit_scores(b, w, 0)
                    for h in range(H):
                        g = h & 1
                        mt = h >> 1
                        e_sb = e_pipe
                        if h + 1 < H:
                            e_pipe = emit_scores(b, w, h + 1)

                        pv = psPV.tile([128, 260], f32, tag="pv")
                        pv_mms = []
                        first = True
                        for kts in range(2):
                            for br in range(2):
                                for qt in range(2):
                                    col = (br * 2 + qt) * 65
                                    pv_mms.append(nc.tensor.matmul(
                                        pv[:, col:col + 65],
                                        e_sb[br][:, kts * 256 + qt * 128:
                                                 kts * 256 + qt * 128 + 128]
                                        ,
                                        vext[br, b, w + kts, g],
                                        start=first,
                                        stop=(kts == 1 and br == 1 and qt == 1)))
                                    first = False
                        order_group(pv_mms)

                        r_sb = smp.tile([128, 4], f32, tag="recip")
                        nc.vector.reciprocal(r_sb, pv[:, 64:260:65])

                        if g == 0:
                            # [128 q, 128] pair tile: h-even dims in cols
                            # 0:64, h-odd in 64:128; transposed in one shot.
                            pair_sb = [pairp.tile([128, 128], bf16,
                                                  tag=f"pair{qt}",
                                                  name=f"pair{qt}")
                                       for qt in range(2)]
                        for qt in range(2):
                            # qt0's scale-copy on ACT, qt1's on DVE: keeps
                            # the two per-head engine loads balanced.
                            t2 = smp.tile([128, 64], bf16, tag="t2")
                            if qt == 0:
                                nc.scalar.activation(
                                    t2,
                                    pv[:, 130 + qt * 65:130 + qt * 65 + 64],
                                    ACTF.Copy, scale=r_sb[:, 2 + qt:3 + qt])
                            else:
                                nc.vector.tensor_scalar(
                                    out=t2,
                                    in0=pv[:, 130 + qt * 65:
                                           130 + qt * 65 + 64],
                                    scalar1=r_sb[:, 2 + qt:3 + qt],
                                    scalar2=None, op0=ALU.mult)
                            nc.vector.scalar_tensor_tensor(
                                out=pair_sb[qt][:, g * 64:(g + 1) * 64],
                                in0=pv[:, qt * 65:qt * 65 + 64],
                                scalar=r_sb[:, qt:qt + 1], in1=t2,
                                op0=ALU.mult, op1=ALU.subtract)
                        if g == 1:
                            # transpose [q, d-pair] -> [d-pair, q] on the
                            # (otherwise idle) DMA engines, freeing the PE
                            # and DVE of 64 transposes + copies.
                            for qt in range(2):
                                at = atp.tile([128, 128], bf16,
                                              tag=f"at{mt}_{qt}")
                                nc.sync.dma_start(out=at, in_=pair_sb[qt],
                                                  transpose=True)
                                at_sb[mt, qt] = at

                        # PE filler between heads: batch-1 projection
                        # groups first, then the previous block's deferred
                        # y-projection groups.
                        if fillers:
                            fillers.pop()()
                        elif prev_y:
                            prev_y.pop(0)()

                    while prev_y:      # any leftovers before next block
                        prev_y.pop(0)()
                    prev_y = [make_y_thunk(b, w, qt, nh, dict(at_sb))
                              for qt in range(2) for nh in range(2)]
            for t in prev_y:           # last block's y runs at the end
                t()
    split_matmul_waits()
    return nc


def get_program():
    if "nc" not in _PROGRAM_CACHE:
        _PROGRAM_CACHE["nc"] = _build_program()
    return _PROGRAM_CACHE["nc"]


# ------------------------------------------------------------------ host API

def make_in_maps(x, Wq1, Wq2, Wk1, Wk2, Wv, Wout, bout, lq1, lk1, lq2, lk2):
    x = np.asarray(x, dtype=np.float32)
    lam = float(np.clip(
        np.exp(np.asarray(lq1, np.float64) @ np.asarray(lk1, np.float64))
        - np.exp(np.asarray(lq2, np.float64) @ np.asarray(lk2, np.float64))
        + LAMBDA_INIT, 0.1, 0.9))

    qp1, qp2 = _head_perm(), _q2_perm()
    kp1, kp2 = _k_perm(False), _k_perm(True)

    wq = np.stack([
        _strip_w(np.asarray(Wq1, np.float32)[:, qp1]),
        _strip_w(np.asarray(Wq2, np.float32)[:, qp2]),
    ]).astype(BF16)  # (2, 8, 128, 1024)
    wk = np.stack([
        _strip_w(np.asarray(Wk1, np.float32)[:, kp1])[0],
        _strip_w(np.asarray(Wk2, np.float32)[:, kp2])[0],
    ]).astype(BF16)  # (2, 128, 1024)
    wv = _strip_w(np.asarray(Wv, np.float32))[0].astype(BF16)  # (128, 1024)
    # wo strip kt: [128 p, 1024 cols] = Wout rows kt*128..+128 (contiguous)
    wo = np.ascontiguousarray(
        np.asarray(Wout, np.float32).reshape(8, 128, 1024)).astype(BF16)
    boutv = np.asarray(bout, np.float32).reshape(1, E)

    lamv = np.zeros((128, 2), np.float32)
    lamv[:, 0] = 1.0
    lamv[:, 1] = 1.0 / lam

    in_maps = []
    for c in range(NCORES):
        s0 = 256 * c
        # xt[b, p, k*384+s] = x[b, s0+s, k*128+p]
        xt = np.ascontiguousarray(
            x[:, s0:s0 + SEQ, :].reshape(B, SEQ, 8, 128)
            .transpose(0, 3, 2, 1).reshape(B, 128, 8 * SEQ)).astype(BF16)
        tct, tst = _trig_tables(c)
        in_maps.append({
            "xt": xt, "wq": wq, "wk": wk, "wv": wv, "wo": wo,
            "tct": tct, "tst": tst, "lamv": lamv, "boutv": boutv,
            "p32": _p32().astype(BF16),
            "onesv": np.ones((1, 128), np.float32),
        })
    return in_maps


def kernel(**inputs) -> np.ndarray:
    from concourse.bass_utils import run_bass_kernel_spmd

    in_maps = make_in_maps(**inputs)
    nc = get_program()
    res = run_bass_kernel_spmd(nc, in_maps, core_ids=list(range(NCORES)))
    out = np.empty((B, L, E), dtype=np.float32)
    for c in range(NCORES):
        out[:, 512 * c:512 * (c + 1), :] = res.results[c]["y"]
    return out


# revision 20
# speedup vs baseline: 1.2620x; 1.2620x over previous
"""Trainium2 Bass kernel for EnhancedLocalAttentionWithGQA (differential
windowed attention, B=2 L=4096 E=1024 H=16 G=2 W=256 D=64).

Key structural facts exploited:
  - The reference concatenates nw=31 overlapping windows along the sequence
    and trims to L=4096 = 16*W, so ONLY windows 0..15 contribute, and only
    input positions 0..2175 are used as queries/keys/values.
  - Output row p comes from window n = p//256, in-window query j = p%256,
    i.e. input position n*128 + j.

Sharding: 8 cores, core c owns windows (2c, 2c+1) -> output rows
[512c, 512c+512) for both batches. Each core needs x rows [256c, 256c+384).

v2: whole matmul pipeline in bf16 (error budget 2e-2, measured ~6.5e-3);
fp32r small-N matmuls ran 4 cycles/row + slow LDWEIGHTS, bf16 is 1
cycle/row.  All weights HBM-resident in bf16 in the exact SBUF strip
layout (fully contiguous per-partition DMA).        [436852 -> 236862 ns]

v3: keep the PE p-state at full clock by never letting it idle: batch-1
projections are emitted as filler between batch-0 attention heads, score
matmuls are software-pipelined one head ahead of PV, rope's second
multiply and the t2 scale move to the idle GpSimd engine, and the first
xt/wq DMAs are split into chunks so the first projection starts ~8us
earlier.

On-device dataflow (per core, SPMD — all core differences come via inputs):
  - q^T / k^T computed directly in [head-dim, seq] layout (lhsT = weight
    tile, rhs = x^T tile). Host pre-permutes W columns so each head block
    is [evens | odds], making RoPE 2 full-tile muls + 4 partition-offset
    add/subs. Branch-2 weights are block-swapped so the two differential
    branches occupy complementary 64-partition halves (concurrent K=64
    score matmuls via PE row groups).
  - Scores computed transposed: S^T[k, q] (lhsT = k^T, rhs = q^T), exp on
    ACT without max-subtraction (scores are small), then PV as
    out[q, 65] = E_tile.T @ [v | ones-ish] with an extra column giving the
    softmax denominator (branch 2 uses 1/lambda so its reciprocal is
    lambda/den2).
  - Normalize+combine: one GpSimd scale (t2 = num2*r2) + one DVE
    scalar_tensor_tensor -> a = num1*r1 - t2.
  - a transposed back via PE transpose (head pair packed into one PSUM
    tile), out-projection accumulated over 8 K-tiles + bias via K=1 matmul.
"""

import os
import sys

sys.path.insert(0, "/opt/trn_rl_repo")
os.environ.setdefault("MYCRO_LOCAL_CACHE", "1")

import numpy as np
import ml_dtypes

BF16 = ml_dtypes.bfloat16

B, L, E, H, G, W, D = 2, 4096, 1024, 16, 2, 256, 64
NCORES = 8
SEQ = 384          # x rows per core
NW = 2             # windows per core
QROWS = 512        # output rows per core per batch
KV = E // (H // G)  # 128
LAMBDA_INIT = 0.8


# ----------------------------------------------------------------- host prep

def _head_perm():
    """Column permutation applied to Wq1/Wk1: per 64-block -> [evens|odds]."""
    p = []
    for blk in range(0, E, D):
        p += [blk + 2 * j for j in range(D // 2)]
        p += [blk + 2 * j + 1 for j in range(D // 2)]
    return np.array(p, dtype=np.int64)


def _q2_perm():
    """q2: like _head_perm but heads swapped within each 128-col M-tile."""
    base = _head_perm()
    p = np.empty_like(base)
    for m in range(E // 128):
        p[m * 128: m * 128 + 64] = base[m * 128 + 64: m * 128 + 128]
        p[m * 128 + 64: m * 128 + 128] = base[m * 128: m * 128 + 64]
    return p


def _k_perm(swap):
    """kv columns (128 = 2 groups x 64): per group block [evens|odds];
    swap=True puts group1 first (branch-2 layout)."""
    p = []
    groups = (1, 0) if swap else (0, 1)
    for g in groups:
        blk = g * D
        p += [blk + 2 * j for j in range(D // 2)]
        p += [blk + 2 * j + 1 for j in range(D // 2)]
    return np.array(p, dtype=np.int64)


def _strip_w(w):
    """(1024, M) -> (M//128 mt, 128 p, 8k*128) strips: strip[mt][p][k*128+s]
    = w[k*128+p, mt*128+s]; per-partition rows are fully contiguous."""
    kdim = w.shape[0] // 128
    mdim = w.shape[1] // 128
    t = w.reshape(kdim, 128, mdim, 128).transpose(2, 1, 0, 3)
    return np.ascontiguousarray(t.reshape(mdim, 128, kdim * 128))


def _trig_tables(core):
    pos = (256 * core + np.arange(SEQ, dtype=np.float64))  # global positions
    div = np.exp(np.arange(0, D, 2, dtype=np.float64) * (-np.log(10000.0) / D))
    ang = pos[None, :] * div[:, None]          # (32, SEQ)
    c32 = np.cos(ang).astype(np.float32)
    s32 = np.sin(ang).astype(np.float32)
    tc = np.tile(c32, (4, 1))                   # (128, SEQ)
    # sign-folded sin: rows [0:32]=+sin (qe*sin for the odd half),
    # [32:64]=-sin (-qo*sin for the even half), repeating per 64-block.
    tsn = np.tile(np.concatenate([s32, -s32], axis=0), (2, 1))
    return np.ascontiguousarray(tc), np.ascontiguousarray(tsn)


def _p32():
    """[128,128] permutation: swaps 32-halves within each 64-block.
    Used as matmul lhsT: out = P.T @ u with P[k, m] = 1 iff k = swap(m)."""
    p = np.zeros((128, 128), np.float32)
    for m in range(128):
        k = m + 32 if (m % 64) < 32 else m - 32
        p[k, m] = 1.0
    return p


# ------------------------------------------------------------ device program

_PROGRAM_CACHE = {}


def _build_program():
    import concourse.bass as bass
    import concourse.mybir as mybir
    import concourse.tile as tile
    from concourse.masks import make_identity
    from concourse.tile_rust import add_dep_helper

    def order_group(insts):
        """PE-order a bank-packed accumulation group: first (start=True)
        before everything, last (stop=True) after everything. sync=False —
        same-engine ordering only."""
        first, last = insts[0], insts[-1]
        for i in insts[1:]:
            add_dep_helper(i.ins, first.ins, sync=False,
                           reason="psum group start first")
        for i in insts[:-1]:
            add_dep_helper(last.ins, i.ins, sync=False,
                           reason="psum group stop last")

    f32 = mybir.dt.float32
    f32r = mybir.dt.float32r
    bf16 = mybir.dt.bfloat16
    ALU = mybir.AluOpType
    ACTF = mybir.ActivationFunctionType

    nc = bass.Bass()

    xt_d = nc.dram_tensor("xt", [B, 128, 8 * SEQ], bf16, kind="ExternalInput")
    wq_d = nc.dram_tensor("wq", [2, 8, 128, 1024], bf16, kind="ExternalInput")
    wk_d = nc.dram_tensor("wk", [2, 128, 1024], bf16, kind="ExternalInput")
    wv_d = nc.dram_tensor("wv", [128, 1024], bf16, kind="ExternalInput")
    wo_d = nc.dram_tensor("wo", [8, 128, 1024], bf16, kind="ExternalInput")
    tc_d = nc.dram_tensor("tct", [128, SEQ], f32, kind="ExternalInput")
    ts_d = nc.dram_tensor("tst", [128, SEQ], f32, kind="ExternalInput")
    lam_d = nc.dram_tensor("lamv", [128, 2], f32, kind="ExternalInput")
    bout_d = nc.dram_tensor("boutv", [1, E], f32r, kind="ExternalInput")
    p32_d = nc.dram_tensor("p32", [128, 128], bf16, kind="ExternalInput")
    ones_d = nc.dram_tensor("onesv", [1, 128], f32r, kind="ExternalInput")
    y_d = nc.dram_tensor("y", [B, QROWS, E], f32, kind="ExternalOutput")

    def split_matmul_waits():
        """This walrus build allows only ONE sync-wait per engine
        instruction; peel extra waits onto engine-matched no-ops placed
        just before the instruction."""
        for bb in nc.m.functions[0].blocks:
            il = bb.instructions
            new_list = []
            changed = False
            for i in il:
                si = getattr(i, "sync_info", None)
                if si is not None and len(si.on_wait) > 1:
                    waits = list(si.on_wait)
                    for j, w in enumerate(waits[1:]):
                        nop = mybir.InstNoOp(
                            name=f"{i.name}-wnop{j}", engine=i.engine, ins=[],
                            outs=[],
                            sync_info=mybir.SyncInfo(on_wait=[w],
                                                     on_update=[]))
                        nc.inst_map[nop.name] = nop
                        new_list.append(nop)
                    i.sync_info = mybir.SyncInfo(
                        on_wait=[waits[0]], on_update=list(si.on_update))
                    changed = True
                new_list.append(i)
            if changed:
                il[:] = new_list
    with tile.TileContext(nc) as tc:
        with tc.tile_pool(name="const", bufs=1) as constp, \
             tc.tile_pool(name="xt", bufs=1) as xtp, \
             tc.tile_pool(name="rot", bufs=1) as rotp, \
             tc.tile_pool(name="wres", bufs=1) as wresp, \
             tc.tile_pool(name="ru", bufs=3) as rup, \
             tc.tile_pool(name="vext", bufs=1) as vxp, \
             tc.tile_pool(name="att", bufs=4) as attp, \
             tc.tile_pool(name="small", bufs=3) as smp, \
             tc.tile_pool(name="pairs", bufs=6) as pairp, \
             tc.tile_pool(name="atile", bufs=2) as atp, \
             tc.tile_pool(name="psA", bufs=2, space="PSUM") as psA, \
             tc.tile_pool(name="psSC", bufs=3, space="PSUM") as psSC, \
             tc.tile_pool(name="psPV", bufs=3, space="PSUM") as psPV:

            # constants
            ones1 = constp.tile([1, 128], f32r, tag="ones1")
            nc.sync.dma_start(out=ones1, in_=ones_d[:, :])
            p32_sb = constp.tile([128, 128], bf16, tag="p32s")
            nc.sync.dma_start(out=p32_sb, in_=p32_d[:, :])
            tc_sb = constp.tile([128, SEQ], f32, tag="tcs")
            ts_sb = constp.tile([128, SEQ], f32, tag="tss")
            lam_sb = constp.tile([128, 2], f32, tag="lams")
            bout_sb = constp.tile([1, E], f32r, tag="bouts")
            nc.sync.dma_start(out=tc_sb, in_=tc_d[:, :])
            nc.sync.dma_start(out=ts_sb, in_=ts_d[:, :])
            nc.sync.dma_start(out=lam_sb, in_=lam_d[:, :])
            nc.sync.dma_start(out=bout_sb, in_=bout_d[:, :])

            # x^T strips.  b0's strip and the first q weight strip are
            # split into chunks and interleaved so the very first
            # projection group can start after ~1/4 of the data landed.
            xts = {}
            strips = {}
            for b in range(B):
                strips[b] = xtp.tile([128, 8 * SEQ], bf16, tag=f"xt{b}",
                                     name=f"xt{b}")
                for kt in range(8):
                    xts[b, kt] = strips[b][:, kt * SEQ:(kt + 1) * SEQ]
            wq_sb = {}
            wq_sb[0, 0] = wresp.tile([128, 1024], bf16, tag="wq_0_0",
                                     name="wq_0_0")
            for j in range(4):
                nc.sync.dma_start(out=strips[0][:, j * 768:(j + 1) * 768],
                                  in_=xt_d[0, :, j * 768:(j + 1) * 768])
                nc.sync.dma_start(
                    out=wq_sb[0, 0][:, j * 256:(j + 1) * 256],
                    in_=wq_d[0, 0, :, j * 256:(j + 1) * 256])
            # resident weights, bf16, contiguous strips.  The next few q
            # strips go before xt batch-1 (they gate the projection
            # pipeline; batch-1 data is not needed until the attention
            # phase).
            for mat in range(2):
                for mt in range(8):
                    if (mat, mt) == (0, 0):
                        continue
                    t = wresp.tile([128, 1024], bf16, tag=f"wq_{mat}_{mt}")
                    nc.sync.dma_start(out=t, in_=wq_d[mat, mt, :, :])
                    wq_sb[mat, mt] = t
                    if (mat, mt) == (0, 4):
                        nc.sync.dma_start(out=strips[1], in_=xt_d[1, :, :])
            wk_sb = {}
            for mat in range(2):
                wk_sb[mat] = wresp.tile([128, 1024], bf16, tag=f"wk_{mat}",
                                        name=f"wk_{mat}")
                nc.sync.dma_start(out=wk_sb[mat], in_=wk_d[mat, :, :])
            wv_sb = wresp.tile([128, 1024], bf16, tag="wv")
            nc.sync.dma_start(out=wv_sb, in_=wv_d[:, :])
            wo_sb = {}
            for kt in range(8):
                t = wresp.tile([128, 1024], bf16, tag=f"wo{kt}")
                nc.sync.dma_start(out=t, in_=wo_d[kt, :, :])
                for nh in range(2):
                    wo_sb[kt, nh] = t[:, nh * 512:(nh + 1) * 512]

            qrot = {}
            krot = {}
            vext = {}

            def rope(psum_in, rot_out):
                # rot = psum*TC + P32 @ (psum*TS_signed)
                # (the PE matmul does the cross-partition 32-half swap that
                # DVE cannot: walrus requires same start partition on all
                # InstTensorTensor operands; GpSimd cannot access PSUM)
                t = rup.tile([128, SEQ], bf16, tag="ropet")
                u = rup.tile([128, SEQ], bf16, tag="ropeu")
                nc.vector.tensor_mul(t, psum_in, tc_sb)
                nc.vector.tensor_mul(u, psum_in, ts_sb)
                usw = psSC.tile([128, 512], f32, tag="sc")
                nc.tensor.matmul(usw[:, 0:SEQ], p32_sb,
                                 u, start=True, stop=True)
                nc.vector.tensor_add(rot_out, t, usw[:, 0:SEQ])

            def emit_qproj(mat, mt, b):
                ps = psA.tile([128, 512], f32, tag="proj")
                qp = ps[:, 0:SEQ]
                for kt in range(8):
                    nc.tensor.matmul(
                        qp,
                        wq_sb[mat, mt][:, kt * 128:(kt + 1) * 128],
                        xts[b, kt],
                        start=(kt == 0), stop=(kt == 7))
                rot = rotp.tile([128, SEQ], bf16, tag=f"q{mat}_{b}_{mt}",
                                name=f"q{mat}_{b}_{mt}")
                rope(qp, rot)
                qrot[mat, b, mt] = rot

            def emit_kproj(mat, b):
                ps = psA.tile([128, 512], f32, tag="proj")
                kp = ps[:, 0:SEQ]
                for kt in range(8):
                    nc.tensor.matmul(
                        kp, wk_sb[mat][:, kt * 128:(kt + 1) * 128],
                        xts[b, kt],
                        start=(kt == 0), stop=(kt == 7))
                rot = rotp.tile([128, SEQ], bf16, tag=f"k{mat}_{b}",
                                name=f"k{mat}_{b}")
                rope(kp, rot)
                krot[mat, b] = rot

            def emit_vproj(b, st):
                ps = psA.tile([128, 512], f32, tag="proj")
                vp = ps[:, 0:128]
                for kt in range(8):
                    nc.tensor.matmul(
                        vp,
                        xts[b, kt][:, st * 128:(st + 1) * 128],
                        wv_sb[:, kt * 128:(kt + 1) * 128],
                        start=(kt == 0), stop=(kt == 7))
                for g in range(2):
                    for ver in range(2):
                        ve = vxp.tile([128, 65], bf16,
                                      tag=f"ve{ver}_{b}_{st}_{g}",
                                      name=f"ve{ver}_{b}_{st}_{g}")
                        nc.vector.tensor_copy(ve[:, 0:64],
                                              vp[:, g * 64:(g + 1) * 64])
                        nc.gpsimd.tensor_copy(ve[:, 64:65],
                                              lam_sb[:, ver:ver + 1])
                        vext[ver, b, st, g] = ve

            # ---- batch-0 projections ----
            for mat in range(2):
                for mt in range(8):
                    emit_qproj(mat, mt, 0)
            for mat in range(2):
                emit_kproj(mat, 0)
            for st in range(3):
                emit_vproj(0, st)

            # batch-1 projection thunks: emitted as PE filler between
            # batch-0 attention heads (keeps the PE p-state at full clock
            # and moves their DVE rope work into the ACT-heavy phase).
            fillers = [(lambda mat=mat, mt=mt: emit_qproj(mat, mt, 1))
                       for mat in range(2) for mt in range(8)]
            fillers += [(lambda mat=mat: emit_kproj(mat, 1))
                        for mat in range(2)]
            fillers += [(lambda st=st: emit_vproj(1, st)) for st in range(3)]
            fillers.reverse()  # pop() order = original order

            # ---- attention + output projection ----
            def emit_scores(b, w, h):
                g = h & 1
                mt = h >> 1
                base1 = 64 * g          # branch-1 partition base
                base2 = 64 - base1      # branch-2 partition base
                e_sb = []
                for br, qb in ((0, base1), (1, base2)):
                    st_ps = psSC.tile([128, 512], f32, tag="sc")
                    sc_mms = []
                    for kts in range(2):
                        sc_mms.append(nc.tensor.matmul(
                            st_ps[:, kts * 256:(kts + 1) * 256],
                            krot[br, b][qb:qb + 64,
                                        w * 128 + kts * 128:
                                        w * 128 + kts * 128 + 128]
                            ,
                            qrot[br, b, mt][qb:qb + 64,
                                            w * 128:w * 128 + 256]
                            ,
                            start=(kts == 0), stop=(kts == 1)))
                    order_group(sc_mms)
                    e = attp.tile([128, 512], bf16, tag=f"e{br}")
                    nc.scalar.activation(e, st_ps, ACTF.Exp,
                                         scale=0.125)
                    e_sb.append(e)
                return e_sb

            def make_y_thunk(b, w, qt, nh, at_map):
                def thunk():
                    y_ps = psA.tile([128, 512], f32, tag="proj")
                    y_mms = []
                    for kt in range(8):
                        y_mms.append(nc.tensor.matmul(
                            y_ps, at_map[kt, qt],
                            wo_sb[kt, nh],
                            start=(kt == 0), stop=False))
                    y_mms.append(nc.tensor.matmul(
                        y_ps, ones1,
                        bout_sb[:, nh * 512:(nh + 1) * 512],
                        start=False, stop=True))
                    order_group(y_mms)
                    y_sb = smp.tile([128, 512], f32, tag="ysb")
                    if nh == 0:
                        nc.scalar.activation(y_sb, y_ps, ACTF.Copy)
                    else:
                        nc.vector.tensor_copy(y_sb, y_ps)
                    nc.sync.dma_start(
                        out=y_d[b, (w * 2 + qt) * 128:
                                (w * 2 + qt) * 128 + 128,
                                nh * 512:(nh + 1) * 512],
                        in_=y_sb)
                return thunk

            for b in range(B):
                for w in range(NW):
                    at_sb = {}
                    pair_sb = None
                    e_pipe = emit_scores(b, w, 0)
                    for h in range(H):
                        g = h & 1
                        mt = h >> 1
                        e_sb = e_pipe
                        if h + 1 < H:
                            e_pipe = emit_scores(b, w, h + 1)

                        pv = psPV.tile([128, 260], f32, tag="pv")
                        pv_mms = []
                        first = True
                        for kts in range(2):
                            for br in range(2):
                                for qt in range(2):
                                    col = (br * 2 + qt) * 65
                                    pv_mms.append(nc.tensor.matmul(
                                        pv[:, col:col + 65],
                                        e_sb[br][:, kts * 256 + qt * 128:
                                                 kts * 256 + qt * 128 + 128]
                                        ,
                                        vext[br, b, w + kts, g],
                                        start=first,
                                        stop=(kts == 1 and br == 1 and qt == 1)))
                                    first = False
                        order_group(pv_mms)

                        r_sb = smp.tile([128, 4], f32, tag="recip")
                        nc.vector.reciprocal(r_sb, pv[:, 64:260:65])

                        if g == 0:
                            # [128 q, 128] pair tile: h-even dims in cols
                            # 0:64, h-odd in 64:128; transposed in one shot.
                            pair_sb = [pairp.tile([128, 128], bf16,
                                                  tag=f"pair{qt}",
                                                  name=f"pair{qt}")
                                       for qt in range(2)]
                        for qt in range(2):
                            # qt0's scale-copy on ACT, qt1's on DVE: keeps
                            # the two per-head engine loads balanced.
                            t2 = smp.tile([128, 64], bf16, tag="t2")
                            if qt == 0:
                                nc.scalar.activation(
                                    t2,
                                    pv[:, 130 + qt * 65:130 + qt * 65 + 64],
                                    ACTF.Copy, scale=r_sb[:, 2 + qt:3 + qt])
                            else:
                                nc.vector.tensor_scalar(
                                    out=t2,
                                    in0=pv[:, 130 + qt * 65:
                                           130 + qt * 65 + 64],
                                    scalar1=r_sb[:, 2 + qt:3 + qt],
                                    scalar2=None, op0=ALU.mult)
                            nc.vector.scalar_tensor_tensor(
                                out=pair_sb[qt][:, g * 64:(g + 1) * 64],
                                in0=pv[:, qt * 65:qt * 65 + 64],
                                scalar=r_sb[:, qt:qt + 1], in1=t2,
                                op0=ALU.mult, op1=ALU.subtract)
                        if g == 1:
                            # transpose [q, d-pair] -> [d-pair, q] on the
                            # (otherwise idle) DMA engines, freeing the PE
                            # and DVE of 64 transposes + copies.
                            for qt in range(2):
                                at = atp.tile([128, 128], bf16,
                                              tag=f"at{mt}_{qt}")
                                nc.sync.dma_start(out=at, in_=pair_sb[qt],
                                                  transpose=True)
                                at_sb[mt, qt] = at

                        # PE filler between heads: batch-1 projection
                        # groups during batch-0 attention.
                        if b == 0 and fillers:
                            fillers.pop()()

                    for qt in range(2):
                        for nh in range(2):
                            make_y_thunk(b, w, qt, nh, at_sb)()
    split_matmul_waits()
    return nc


def get_program():
    if "nc" not in _PROGRAM_CACHE:
        _PROGRAM_CACHE["nc"] = _build_program()
    return _PROGRAM_CACHE["nc"]


# ------------------------------------------------------------------ host API

def make_in_maps(x, Wq1, Wq2, Wk1, Wk2, Wv, Wout, bout, lq1, lk1, lq2, lk2):
    x = np.asarray(x, dtype=np.float32)
    lam = float(np.clip(
        np.exp(np.asarray(lq1, np.float64) @ np.asarray(lk1, np.float64))
        - np.exp(np.asarray(lq2, np.float64) @ np.asarray(lk2, np.float64))
        + LAMBDA_INIT, 0.1, 0.9))

    qp1, qp2 = _head_perm(), _q2_perm()
    kp1, kp2 = _k_perm(False), _k_perm(True)

    wq = np.stack([
        _strip_w(np.asarray(Wq1, np.float32)[:, qp1]),
        _strip_w(np.asarray(Wq2, np.float32)[:, qp2]),
    ]).astype(BF16)  # (2, 8, 128, 1024)
    wk = np.stack([
        _strip_w(np.asarray(Wk1, np.float32)[:, kp1])[0],
        _strip_w(np.asarray(Wk2, np.float32)[:, kp2])[0],
    ]).astype(BF16)  # (2, 128, 1024)
    wv = _strip_w(np.asarray(Wv, np.float32))[0].astype(BF16)  # (128, 1024)
    # wo strip kt: [128 p, 1024 cols] = Wout rows kt*128..+128 (contiguous)
    wo = np.ascontiguousarray(
        np.asarray(Wout, np.float32).reshape(8, 128, 1024)).astype(BF16)
    boutv = np.asarray(bout, np.float32).reshape(1, E)

    lamv = np.zeros((128, 2), np.float32)
    lamv[:, 0] = 1.0
    lamv[:, 1] = 1.0 / lam

    in_maps = []
    for c in range(NCORES):
        s0 = 256 * c
        # xt[b, p, k*384+s] = x[b, s0+s, k*128+p]
        xt = np.ascontiguousarray(
            x[:, s0:s0 + SEQ, :].reshape(B, SEQ, 8, 128)
            .transpose(0, 3, 2, 1).reshape(B, 128, 8 * SEQ)).astype(BF16)
        tct, tst = _trig_tables(c)
        in_maps.append({
            "xt": xt, "wq": wq, "wk": wk, "wv": wv, "wo": wo,
            "tct": tct, "tst": tst, "lamv": lamv, "boutv": boutv,
            "p32": _p32().astype(BF16),
            "onesv": np.ones((1, 128), np.float32),
        })
    return in_maps


def kernel(**inputs) -> np.ndarray:
    from concourse.bass_utils import run_bass_kernel_spmd

    in_maps = make_in_maps(**inputs)
    nc = get_program()
    res = run_bass_kernel_spmd(nc, in_maps, core_ids=list(range(NCORES)))
    out = np.empty((B, L, E), dtype=np.float32)
    for c in range(NCORES):
        out[:, 512 * c:512 * (c + 1), :] = res.results[c]["y"]
    return out
